# revision 11
# baseline (speedup 1.0000x reference)
"""Trainium2 Bass kernel for nn_CustomLoss_34711925686778.

Data-parallel over the batch axis: B=16384 rows split across 8 NeuronCores
(2048 rows each).  Inputs are downcast to bf16 on the host (the loss is
dominated by the KL term at ~4.1e7 with rel-tol 2e-2, i.e. an absolute
budget of ~8e5; bf16 rounding perturbs the KL mean by O(100)).  Each core
streams its shard from HBM (18.6 MB vs 37.3 MB in f32), computes per-row
partial sums for the four TUBE terms plus global CE/KL partials, and
writes a [128, 8] tile of per-partition partial sums.  The host sums the
partials and applies the final means/scales.

Layout: row r of a core's shard maps to (partition p = r // 16,
group g = r % 16) so every DMA is contiguous per partition.

Engine budget per core (measured on HW):
  DVE  STT fused mul+rowsum [128,512] bf16   ~605 ns (1x: no STT 2x uop)
  DVE  TT mult bf16 (2x) + TS accum          measured via bench.py
  ACT  Square+accum [128,512]                ~820 ns (dtype-independent)
  ACT  table load (Square vs Exp/Ln sets)    ~1.3 us -> group by set
  DMA  18.6 MB @ ~320 GB/s across 16 engines ~46 us busy

Self-contained: hardcodes shapes/sharding; only needs the concourse
toolchain at /opt/trn_rl_repo.
"""

import sys

if "/opt/trn_rl_repo" not in sys.path:
    sys.path.insert(0, "/opt/trn_rl_repo")

import ml_dtypes
import numpy as np

import concourse.bacc as bacc
import concourse.bass as bass
import concourse.mybir as mybir
import concourse.tile as tile
from concourse.bass_utils import run_bass_kernel_spmd

# ---- problem constants (hardcoded from the reference) ----
B, C, D, Z = 16384, 100, 512, 128
L1, L2, ALPHA, BETA, EPS = 0.5, 1.5, 1.0, 50000000.0, 1e-08

NCORES = 8
R = B // NCORES          # 2048 rows per core
P = 128                  # SBUF partitions
G = R // P               # 16 rows per partition
H = G // 2               # groups per half-tensor slab
NPAIR = 4

PAIRS = [
    ("x_A_reconstructed", "x_A"),
    ("x_B_reconstructed", "x_B"),
    ("x_C_reconstructed", "x_C"),
    ("comple_out", "labels_encoder"),
]

# labels stays f32 so the row-max tie-breaking matches the reference argmax
# exactly; everything else is bf16.
BF16 = ml_dtypes.bfloat16
INPUT_SPECS = {
    "fusion_out": (C, BF16),
    "comple_out": (D, BF16),
    "labels": (C, np.float32),
    "labels_encoder": (D, BF16),
    "x_A": (D, BF16),
    "x_A_reconstructed": (D, BF16),
    "x_B": (D, BF16),
    "x_B_reconstructed": (D, BF16),
    "x_C": (D, BF16),
    "x_C_reconstructed": (D, BF16),
    "mu": (Z, BF16),
    "logvar": (Z, BF16),
}

OUT_NAME = "loss_partials"

f32 = mybir.dt.float32
bf16 = mybir.dt.bfloat16
AF = mybir.ActivationFunctionType
ALU = mybir.AluOpType
AX = mybir.AxisListType

# --- tuning knobs ---
# Per half-slab of 8 groups there are 16 square-stats; ACT takes
# SQ_ACT_PER_HALF of them (direct Square+accum), DVE the rest.
USE_TT_TS = False        # bench: TS/STT accum are both 1x; fused STT wins
SQ_ACT_PER_HALF = {0: 10, 1: 10, 2: 10, 3: 10}

_CACHE = {}


def _emit(tc, ins, out_ap):
    nc = tc.nc

    with (
        tc.tile_pool(name="persist", bufs=1) as persist,
        tc.tile_pool(name="prod", bufs=2) as prodp,
        tc.tile_pool(name="scr", bufs=3) as scr,
        tc.tile_pool(name="scr_act", bufs=3) as scr_act,
        tc.tile_pool(name="scrbig", bufs=1) as scrbig,
        tc.tile_pool(name="stats", bufs=1) as stats,
    ):
        # ---- persistent tiles, one per input tensor ----
        def big_tile(name):
            return persist.tile([P, G * D], bf16, tag=name, name=name)

        pair_tiles = [(big_tile(an), big_tile(bn)) for an, bn in PAIRS]
        t_fus = persist.tile([P, G * C], bf16, tag="fusion_out")
        t_labs = persist.tile([P, G * C], f32, tag="labels")
        t_mu = persist.tile([P, G * Z], bf16, tag="mu")
        t_lv = persist.tile([P, G * Z], bf16, tag="logvar")

        def dma_half(t, name, w, h):
            # rows p*16 + (h*8 + j)  ->  partition p, contiguous 8*w elems
            src = ins[name].rearrange("(p g) w -> p g w", g=G)
            dst = t[:, h * H * w : (h + 1) * H * w]
            nc.sync.dma_start(
                dst.rearrange("p (g w) -> p g w", w=w),
                src[:, h * H : (h + 1) * H, :],
            )

        def dma_full(t, name, w):
            nc.sync.dma_start(
                t[:], ins[name].rearrange("(p g) w -> p (g w)", g=G)
            )

        # DMA issue order == compute consumption order: first half of pair0
        # first (phase A starts earliest), then the small CE/KL tensors.
        dma_half(pair_tiles[0][0], PAIRS[0][0], D, 0)
        dma_half(pair_tiles[0][1], PAIRS[0][1], D, 0)
        dma_full(t_mu, "mu", Z)
        dma_full(t_lv, "logvar", Z)
        dma_half(pair_tiles[0][0], PAIRS[0][0], D, 1)
        dma_half(pair_tiles[0][1], PAIRS[0][1], D, 1)
        dma_full(t_fus, "fusion_out", C)
        dma_full(t_labs, "labels", C)
        for pi in range(1, NPAIR):
            for h in range(2):
                dma_half(pair_tiles[pi][0], PAIRS[pi][0], D, h)
                dma_half(pair_tiles[pi][1], PAIRS[pi][1], D, h)

        # ---- stat tiles ----
        dot_all = stats.tile([P, NPAIR * G], f32, tag="dot_all")
        p2_all = stats.tile([P, NPAIR * G], f32, tag="p2_all")
        g2_all = stats.tile([P, NPAIR * G], f32, tag="g2_all")
        lv_sum = stats.tile([P, 1], f32, tag="lv_sum")
        musq_sum = stats.tile([P, 1], f32, tag="musq_sum")
        elv_sum = stats.tile([P, 1], f32, tag="elv_sum")
        esum_ce = stats.tile([P, G], f32, tag="esum_ce")
        labmax = stats.tile([P, G], f32, tag="labmax")
        picked = stats.tile([P, G], f32, tag="picked")

        # ACT stream part 1 (Square table set): KL musq first (mu arrives
        # first), then phase-A squares.  All Exp/Ln ACT work comes after
        # every Square so the ACT table set loads exactly twice.
        s_musq = scrbig.tile([P, G * Z], bf16, tag="kl_musq")
        nc.scalar.activation(s_musq[:], t_mu[:], AF.Square, accum_out=musq_sum[:])

        def emit_half_phase_a(pi, h):
            """Stats for groups [h*H, (h+1)*H) of pair pi."""
            ta, tb = pair_tiles[pi]
            g0 = h * H
            asl = ta[:, g0 * D : (g0 + H) * D]
            bsl = tb[:, g0 * D : (g0 + H) * D]
            if USE_TT_TS:
                # dot: one TT mult (2x) over the half-slab, then TS accums
                pr = prodp.tile([P, H * D], bf16, tag="prod", name="prod")
                nc.vector.tensor_tensor(out=pr[:], in0=asl, in1=bsl, op=ALU.mult)
                for j in range(H):
                    g = g0 + j
                    s = scr.tile([P, D], bf16, tag="ts_dot", name="ts_dot")
                    nc.vector.tensor_scalar(
                        out=s[:], in0=pr[:, j * D : (j + 1) * D],
                        scalar1=1.0, scalar2=0.0, op0=ALU.mult, op1=ALU.add,
                        accum_out=dot_all[:, pi * G + g : pi * G + g + 1],
                    )
            nsq = 0
            for j in range(H):
                g = g0 + j
                if not USE_TT_TS:
                    sd = scr.tile([P, D], bf16, tag="stt_dot", name="stt_dot")
                    nc.vector.scalar_tensor_tensor(
                        out=sd[:], in0=ta[:, g * D : (g + 1) * D], scalar=1.0,
                        in1=tb[:, g * D : (g + 1) * D],
                        op0=ALU.mult, op1=ALU.mult,
                        accum_out=dot_all[:, pi * G + g : pi * G + g + 1],
                    )
                for src, acc in ((ta, p2_all), (tb, g2_all)):
                    sg = src[:, g * D : (g + 1) * D]
                    accap = acc[:, pi * G + g : pi * G + g + 1]
                    if nsq < SQ_ACT_PER_HALF[pi]:
                        ssq = scr_act.tile([P, D], bf16, tag="act_sq", name="act_sq")
                        nc.scalar.activation(
                            ssq[:], sg, AF.Square, accum_out=accap
                        )
                    elif USE_TT_TS:
                        pr = prodp.tile([P, D], bf16, tag="sq_tt", name="sq_tt")
                        nc.vector.tensor_tensor(out=pr[:], in0=sg, in1=sg, op=ALU.mult)
                        s = scr.tile([P, D], bf16, tag="ts_sq", name="ts_sq")
                        nc.vector.tensor_scalar(
                            out=s[:], in0=pr[:], scalar1=1.0, scalar2=0.0,
                            op0=ALU.mult, op1=ALU.add, accum_out=accap,
                        )
                    else:
                        ssq = scr.tile([P, D], bf16, tag="dve_sq", name="dve_sq")
                        nc.vector.scalar_tensor_tensor(
                            out=ssq[:], in0=sg, scalar=1.0, in1=sg,
                            op0=ALU.mult, op1=ALU.mult, accum_out=accap,
                        )
                    nsq += 1

        emit_half_phase_a(0, 0)

        # KL lv sum on DVE (tensor_scalar + accum)
        s_lv = scrbig.tile([P, G * Z], bf16, tag="kl_lv")
        nc.vector.tensor_scalar(
            out=s_lv[:], in0=t_lv[:], scalar1=1.0, scalar2=0.0,
            op0=ALU.mult, op1=ALU.add, accum_out=lv_sum[:],
        )

        emit_half_phase_a(0, 1)

        # CE part 1 on DVE (label row max) — fus/labs arrive early
        lab3 = t_labs[:].rearrange("p (g c) -> p g c", c=C)
        fus3 = t_fus[:].rearrange("p (g c) -> p g c", c=C)
        nc.vector.tensor_reduce(labmax[:], lab3, axis=AX.X, op=ALU.max)

        for pi in range(1, NPAIR):
            emit_half_phase_a(pi, 0)
            emit_half_phase_a(pi, 1)
            if pi == 1:
                # CE picked logits (DVE, tiny ops)
                for g in range(G):
                    s4 = scr.tile([P, C], bf16, tag="ce_pick", name="ce_pick")
                    nc.vector.scalar_tensor_tensor(
                        out=s4[:], in0=lab3[:, g, :],
                        scalar=labmax[:, g : g + 1],
                        in1=fus3[:, g, :], op0=ALU.is_equal, op1=ALU.mult,
                        accum_out=picked[:, g : g + 1],
                    )

        # ---- ACT stream part 2: Exp/Ln table set from here on ----
        s_elv = scrbig.tile([P, G * Z], bf16, tag="kl_elv")
        nc.scalar.activation(s_elv[:], t_lv[:], AF.Exp, accum_out=elv_sum[:])
        e_fus = scrbig.tile([P, G * C], bf16, tag="ce_exp")
        nc.scalar.activation(e_fus[:], t_fus[:], AF.Exp)
        nc.vector.tensor_reduce(
            esum_ce[:], e_fus[:].rearrange("p (g c) -> p g c", c=C),
            axis=AX.X, op=ALU.add,
        )
        # ---- phase B: per-row tube math on the packed [P, 64] stats ----
        # Reformulated to touch the ACT Ln table set only twice:
        #   ds = w * (|g2 - dot| + sqrt(p2*g2 - dot^2)) / sqrt(g2)
        #   tube term = -ln(tanh(1/ds))  (exact; Tanh shares the Exp set)
        # with branch weight w in {0.5, 1, 1.5} from sign(g2-dot), sign(dot).
        W = NPAIR * G

        def st(name):
            return stats.tile([P, W], f32, tag=name, name=name)

        d2 = st("d2")
        nc.vector.tensor_sub(d2[:], g2_all[:], dot_all[:])
        m1 = st("m1")
        nc.vector.tensor_mul(m1[:], p2_all[:], g2_all[:])
        m2 = st("m2")
        nc.vector.tensor_mul(m2[:], dot_all[:], dot_all[:])
        q = st("q")
        nc.vector.tensor_sub(q[:], m1[:], m2[:])
        # Ln round 1 (one table switch for both)
        Lq, Lg2 = st("Lq"), st("Lg2")
        nc.scalar.activation(Lq[:], q[:], AF.Ln)
        nc.scalar.activation(Lg2[:], g2_all[:], AF.Ln)
        # Exp round (back to the Exp/Square/Tanh/Abs set)
        rt, gn = st("rt"), st("gn")
        nc.scalar.activation(rt[:], Lq[:], AF.Exp, scale=0.5)
        nc.scalar.activation(gn[:], Lg2[:], AF.Exp, scale=0.5)
        adf = st("adf")
        nc.scalar.activation(adf[:], d2[:], AF.Abs)
        num = st("num")
        nc.vector.tensor_add(num[:], adf[:], rt[:])
        # Branch weight w = 1 - 0.5*[d2<=0... wait: w = 1 + 0.5*([dot<0]-[d2<=0])
        s1m = st("s1m")
        nc.vector.tensor_scalar(
            out=s1m[:], in0=d2[:], scalar1=0.0, scalar2=None, op0=ALU.is_le,
        )
        sdm = st("sdm")
        nc.vector.scalar_tensor_tensor(
            out=sdm[:], in0=dot_all[:], scalar=0.0, in1=s1m[:],
            op0=ALU.is_lt, op1=ALU.subtract,
        )
        wgt = st("wgt")
        nc.vector.tensor_scalar(
            out=wgt[:], in0=sdm[:], scalar1=0.5, scalar2=1.0,
            op0=ALU.mult, op1=ALU.add,
        )
        wn = st("wn")
        nc.vector.tensor_mul(wn[:], wgt[:], num[:])
        Lwn = st("Lwn")
        nc.scalar.activation(Lwn[:], wn[:], AF.Ln)
        rec = st("rec")
        nc.scalar.activation(rec[:], Lwn[:], AF.Exp, scale=-1.0)
        ids = st("ids")
        nc.vector.tensor_mul(ids[:], rec[:], gn[:])
        th = st("th")
        nc.scalar.activation(th[:], ids[:], AF.Tanh)
        # Ln round 2 (final switch): tube ln(tanh) + CE logsumexp together
        Lth = st("Lth")
        nc.scalar.activation(Lth[:], th[:], AF.Ln)
        lnz = stats.tile([P, G], f32, tag="lnz")
        nc.scalar.activation(lnz[:], esum_ce[:], AF.Ln)
        ce2 = stats.tile([P, G], f32, tag="ce2")
        nc.vector.tensor_sub(ce2[:], lnz[:], picked[:])
        ce_col = stats.tile([P, 1], f32, tag="ce_col")
        nc.vector.tensor_reduce(ce_col[:], ce2[:], axis=AX.X, op=ALU.add)

        # ---- assemble [P, 8] output ----
        out_t = stats.tile([P, 8], f32, tag="out")
        nc.vector.tensor_reduce(
            out_t[:, 0:NPAIR],
            Lth[:].rearrange("p (i g) -> p i g", g=G),
            axis=AX.X, op=ALU.add,
        )
        nc.vector.tensor_copy(out_t[:, 4:5], lv_sum[:])
        nc.vector.tensor_copy(out_t[:, 5:6], musq_sum[:])
        nc.vector.tensor_copy(out_t[:, 6:7], elv_sum[:])
        nc.vector.tensor_copy(out_t[:, 7:8], ce_col[:])
        nc.sync.dma_start(out_ap, out_t[:])


def build_nc():
    """Build (once) the Bass module shared by all 8 cores."""
    if "nc" in _CACHE:
        return _CACHE["nc"]
    nc = bacc.Bacc(
        "TRN2", target_bir_lowering=False, debug=False, num_devices=NCORES
    )
    ins = {}
    for name, (w, npdt) in INPUT_SPECS.items():
        dt = f32 if npdt == np.float32 else bf16
        ins[name] = nc.dram_tensor(name, [R, w], dt, kind="ExternalInput").ap()
    out_ap = nc.dram_tensor(OUT_NAME, [P, 8], f32, kind="ExternalOutput").ap()
    with tile.TileContext(nc) as tc:
        _emit(tc, ins, out_ap)
    nc.compile()
    _CACHE["nc"] = nc
    return nc


def make_in_maps(inputs):
    """Slice full inputs into 8 per-core shards and downcast on the host."""
    in_maps = []
    for i in range(NCORES):
        m = {}
        for name, (w, npdt) in INPUT_SPECS.items():
            arr = np.asarray(inputs[name])
            m[name] = np.ascontiguousarray(arr[i * R : (i + 1) * R]).astype(npdt)
        in_maps.append(m)
    return in_maps


def combine(results):
    """Host-side gather: fold per-core [128, 8] partials into the loss."""
    totals = np.zeros(8, dtype=np.float64)
    for res in results:
        totals += res[OUT_NAME].astype(np.float64).sum(axis=0)
    tube_terms = [-totals[i] / B for i in range(NPAIR)]
    kl_mean = 1.0 + (totals[4] - totals[5] - totals[6]) / (B * Z)
    kl = -0.5 * BETA * kl_mean
    ce = totals[7] / B
    loss = (
        ALPHA * (tube_terms[0] + tube_terms[1] + tube_terms[2])
        + kl + ce + ALPHA * tube_terms[3]
    )
    return np.array(loss, dtype=np.float32)


def kernel(**inputs):
    nc = build_nc()
    res = run_bass_kernel_spmd(nc, make_in_maps(inputs), core_ids=list(range(NCORES)))
    return combine(res.results)


if __name__ == "__main__":
    rng = np.random.default_rng(0)
    fake = {
        n: rng.standard_normal((R * NCORES, w)).astype(np.float32)
        for n, (w, _) in INPUT_SPECS.items()
    }
    print(kernel(**fake))
